# revision 44
# baseline (speedup 1.0000x reference)
"""Trainium2 Bass kernel for nn_Decode_Layer (dense transformer decode layer).

Strategy (8 NeuronCores, SPMD, sequence-parallel):
  - Core c owns position chunks {c, 15-c} (128 pos each) of both batches
    -> 512 tokens/core, balanced causal work.  Column order within the 512:
    [b0@c, b0@15-c, b1@c, b1@15-c].
  - Residual stream lives FEATURE-MAJOR in SBUF ([128 feat, 8 dt, 512 tok]);
    host pre-transposes x/memory (free), so no on-device transposes, and the
    output ships feature-major and is transposed back on host.
  - RMS norm: per-token 1/rms commutes through the projections, so it is
    folded into the psum-evacuation multiplies / fp8 input casts.  rms1 is a
    pure input function -> computed on host; rms2/3 use Act squares + a PE
    ones-matmul + ln/exp rsqrt, decoupled from the projection critical
    paths (raw-residual casts; 1/rms applied in the psum sinks).
  - fp8e4m3 DoubleRow matmuls (2 contraction blocks per pass, 0.5 cyc/row)
    for Q/K/V/O projections and attention*V.  Weights host-cast to fp8 with
    a x32 scale folded back out through the evac scalars.  The FFN uses
    hi-lo fp8 decomposition (x = hi + lo fp8; Wh@hi + Wh@lo + Wl@hi via
    DoubleRow, Wh tile reused across passes) -- better than bf16 accuracy
    at 75% of its PE cost; plain fp8 there would breach the 2e-2 gate.
  - Scores bf16; exp outputs fp8 directly; softmax denominator falls out of
    the same AV matmul via a ones column in V (VW=80 head stride keeps the
    dual-fp8 Ldweights 16B-aligned).  Exps read 2-bank psum tiles
    ([128,1024] per Act instruction) to halve instruction overhead.
  - K/V computed per-shard and AllGather'd (K bf16, V fp8); causality and
    cross-attn position bias injected additively into psum via identity
    matmuls (host-built masks; bias pre-shifted by -ln 64 so fp8 exp cannot
    overflow).  Projections run in dt-pairs over 2-bank psums with merged
    evacuations; DMA queues split (pure loads on SP, dependent cc writes on
    Act, collectives on gpsimd) to avoid head-of-line blocking.
"""
import numpy as np

B, L, D, H, HD, FFN = 2, 2048, 1024, 16, 64, 4096
P = 128
NC = 8
DT = D // P            # 8 feature tiles
KP = DT // 2           # 4 contraction pairs (256 each)
KF = FFN // P          # 32 ffn tiles
KFP = KF // 2          # 16 ffn contraction pairs
S = 32.0               # fp8 weight scale (S*S == D so rms fold is free)
VW = 80                # per-head V stride (64 v + ones + pad; 16B-aligned for DR)
EPS = 1e-6
LN64 = float(np.log(64.0))
BLOCKS = [(b, q2) for b in range(B) for q2 in range(2)]

_CACHE = {}


def _build(timeline=False, stages=99):
    import concourse.bacc as bacc
    import concourse.mybir as mybir
    import concourse.tile as tile
    from contextlib import ExitStack

    f32 = mybir.dt.float32
    bf16 = mybir.dt.bfloat16
    fp8 = mybir.dt.float8e4
    AF = mybir.ActivationFunctionType
    ALU = mybir.AluOpType
    PM = mybir.MatmulPerfMode

    nc = bacc.Bacc("TRN2", target_bir_lowering=False, debug=False,
                   num_devices=(1 if timeline else NC))

    # ---------------- I/O ----------------
    x32_in = nc.dram_tensor("x32", [DT, P, 512], f32, kind="ExternalInput")
    x8_in = nc.dram_tensor("x8", [DT, P, 512], fp8, kind="ExternalInput")
    mem8_in = nc.dram_tensor("mem8", [DT, P, 512], fp8, kind="ExternalInput")
    posT_in = nc.dram_tensor("posT", [P, 16, 256], bf16, kind="ExternalInput")
    smk_in = nc.dram_tensor("smk", [P, 2, NC, P], bf16, kind="ExternalInput")
    rinv1_in = nc.dram_tensor("rinv1", [1, 512], f32, kind="ExternalInput")
    identb_in = nc.dram_tensor("identb", [P, P], bf16, kind="ExternalInput")
    rinvT1_in = nc.dram_tensor("rinvT1", [P, 4], f32, kind="ExternalInput")
    WQ = [nc.dram_tensor(n, [DT, P, KP, 2, P], fp8, kind="ExternalInput")
          for n in ("wq1", "wq2")]
    WK = [nc.dram_tensor(n, [DT, P, KP, 2, P], fp8, kind="ExternalInput")
          for n in ("wk1", "wk2")]
    WV = [nc.dram_tensor(n, [2, P, KP, 2, 512], fp8, kind="ExternalInput")
          for n in ("wv1", "wv2")]
    WO = [nc.dram_tensor(n, [DT, P, KP, 2, P], fp8, kind="ExternalInput")
          for n in ("wo1", "wo2")]
    W1 = nc.dram_tensor("w1", [KF, P, 2 * KP, 2, P], fp8, kind="ExternalInput")
    W2 = nc.dram_tensor("w2", [DT, P, 2 * KFP, 2, P], fp8,
                        kind="ExternalInput")
    out = nc.dram_tensor("out", [DT, P, 512], f32, kind="ExternalOutput")

    with tile.TileContext(nc) as tc, ExitStack() as g:
        # ---- global pools
        single = g.enter_context(tc.tile_pool(name="single", bufs=1))
        resid = g.enter_context(tc.tile_pool(name="resid", bufs=2))
        h8p = g.enter_context(tc.tile_pool(name="h8p", bufs=2))
        wlhsp = g.enter_context(tc.tile_pool(name="wlhsp", bufs=6))
        wrhsp = g.enter_context(tc.tile_pool(name="wrhsp", bufs=2))
        psp = g.enter_context(tc.tile_pool(name="psp", bufs=2, space="PSUM"))
        psp2 = g.enter_context(tc.tile_pool(name="psp2", bufs=3, space="PSUM"))
        evacp = g.enter_context(tc.tile_pool(name="evacp", bufs=4))
        vop = g.enter_context(tc.tile_pool(name="vop", bufs=2))
        smallp = g.enter_context(tc.tile_pool(name="smallp", bufs=8))
        normp = g.enter_context(tc.tile_pool(name="normp", bufs=8))
        dram = g.enter_context(tc.tile_pool(name="dram", bufs=1, space="DRAM"))

        # collective buffers (k: [feat, tok] bf16; v: [tok, head*65] fp8)
        cc_k_in = [dram.tile([D, 512], bf16, name=f"cck{i}") for i in range(2)]
        cc_v_in = [dram.tile([512, H * VW], fp8, name=f"ccv{i}") for i in range(2)]
        cc_k_out = [dram.tile([NC * D, 512], bf16, addr_space="Shared",
                              name=f"ccko{i}") for i in range(2)]
        cc_v_out = [dram.tile([NC * 512, H * VW], fp8, addr_space="Shared",
                              name=f"ccvo{i}") for i in range(2)]

        # ---- constants / small loads
        ident_b = single.tile([P, P], bf16)
        nc.gpsimd.dma_start(ident_b[:], identb_in.ap())
        ones_b = single.tile([P, 1], bf16)
        nc.gpsimd.memset(ones_b[:], 1.0)
        epsc = single.tile([1, 1], f32)
        nc.gpsimd.memset(epsc[:], D * EPS)
        lnDc = single.tile([1, 1], f32)
        nc.gpsimd.memset(lnDc[:], 0.5 * float(np.log(D)))
        smk = single.tile([P, 2, NC, P], bf16)
        posT = single.tile([P, 16, 256], bf16)

        # residual-stream tiles
        x32 = resid.tile([P, DT, 512], f32, tag="resid", name="x32")

        # ---- helpers
        def mmslot(shape=(P, 512)):
            return psp.tile(list(shape), f32, tag="pb", name="ps")

        def rms_stats(xf):
            """xf [128, DT, 512] f32 -> rb [128,512] f32 (1/rms bcast).
            rsqrt via bit-trick seed + 2 Newton iters, all on DVE (keeps the
            Act table set at {Exp, Square} -> single table load)."""
            ps = mmslot((1, 512))
            for d in range(DT):
                sq = smallp.tile([P, 512], bf16, tag="sq", name="sq")
                nc.scalar.activation(sq[:], xf[:, d, :], AF.Square)
                nc.tensor.matmul(ps[:], ones_b[:], sq[:], start=(d == 0),
                                 stop=(d == DT - 1))
            lnv = smallp.tile([1, 512], f32, tag="ms", name="lnv")
            nc.scalar.activation(lnv[:], ps[:], AF.Ln, bias=epsc[:])
            rinv = smallp.tile([1, 512], f32, tag="ms", name="rinv")
            nc.scalar.activation(rinv[:], lnv[:], AF.Exp, scale=-0.5,
                                 bias=lnDc[:])
            rb = normp.tile([P, 512], f32, tag="rb", bufs=3, name="rb")
            nc.gpsimd.partition_broadcast(rb[:], rinv[:])
            return rb

        def proj_fm_dr(w_dram, rhs8, sink2, dts=DT, kpn=KP, kseg=None,
                       wpool=None):
            """ps2[dt2] [128 dout, 2, 512 tok] = S * W.T @ rhs8 (DoubleRow),
            two output-dt blocks per psum tile (2 banks).  rhs8 may be a
            callable kp -> [128,2,512] AP; weights stream in kseg-pair
            segments."""
            rhs_of = rhs8 if callable(rhs8) else (
                lambda kp: rhs8[:, 2 * kp:2 * kp + 2, :])
            kseg = kseg or kpn
            pool = wpool or wlhsp
            for dt2 in range(dts // 2):
                ps2 = psp2.tile([P, 2, 512], f32, tag="pb2", name="ps2")
                for s0 in range(0, kpn, kseg):
                    wt = pool.tile([P, 2, kseg, 2, P], fp8, tag="wlhs",
                                   name="wt")
                    nc.sync.dma_start(
                        wt[:], w_dram[2 * dt2:2 * dt2 + 2, :,
                                      s0:s0 + kseg].rearrange(
                            "e p k t m -> p e k t m"))
                    for e in range(2):
                        for kp in range(s0, s0 + kseg):
                            nc.tensor.matmul(ps2[:, e, :],
                                             wt[:, e, kp - s0, :, :],
                                             rhs_of(kp),
                                             start=(kp == 0),
                                             stop=(kp == kpn - 1),
                                             perf_mode=PM.DoubleRow)
                sink2(dt2, ps2)

        def proj_fm_bf(w_dram, rhs_b, sink, dts=DT, dkn=DT, wpool=None):
            """psum[dt] [128 dout, 512 tok] = W[:, dt].T @ rhs_b (bf16)."""
            for dt in range(dts):
                wt = (wpool or wlhsp).tile([P, dkn, P], bf16, tag="wlhs",
                                           name="wtb")
                nc.sync.dma_start(wt[:], w_dram[dt])
                ps = mmslot()
                for dk in range(dkn):
                    nc.tensor.matmul(ps[:], wt[:, dk, :], rhs_b[:, dk, :],
                                     start=(dk == 0), stop=(dk == dkn - 1))
                sink(dt, ps)

        def proj_tm_dr(wv_dram, lhs8, sink):
            """ps2[qb] [128 tok, 2(fh), 512 dout] = S * lhs8.T @ Wv (DR)."""
            wrs = []
            for fh in range(2):
                wr = wrhsp.tile([P, KP, 2, 512], fp8, tag="wrhs", name="wr")
                nc.sync.dma_start(wr[:], wv_dram[fh])
                wrs.append(wr)
            for qb in range(4):
                ps2 = psp2.tile([P, 2, 512], f32, tag="pb2", name="vps2")
                for fh in range(2):
                    for kp in range(KP):
                        nc.tensor.matmul(
                            ps2[:, fh, :],
                            lhs8[:, 2 * kp:2 * kp + 2, qb * P:(qb + 1) * P],
                            wrs[fh][:, kp, :, :], start=(kp == 0),
                            stop=(kp == KP - 1), perf_mode=PM.DoubleRow)
                sink(qb, ps2)

        def kv_shard(blk, rhs8, rb, rinvT):
            """K/V shard projections for block blk (rb/rinvT None => 1/S)."""
            kview = cc_k_in[blk][:].rearrange("(d2 e p) t -> d2 e p t", e=2,
                                              p=P)
            vview = cc_v_in[blk][:].rearrange("(q p) f -> q p f", p=P)
            def k_sink(dt2, ps2):
                ev2 = evacp.tile([P, 2, 512], bf16, tag="ev", name="kev")
                for e in range(2):
                    if rb is not None:
                        nc.vector.scalar_tensor_tensor(
                            ev2[:, e, :], ps2[:, e, :], 1.0 / S, rb[:],
                            op0=ALU.mult, op1=ALU.mult)
                    else:
                        nc.vector.tensor_scalar_mul(ev2[:, e, :], ps2[:, e, :],
                                                    1.0 / S)
                for e in range(2):
                    nc.scalar.dma_start(kview[dt2, e], ev2[:, e, :])

            proj_fm_dr(WK[blk], rhs8, k_sink)

            vown = [None] * 4
            for qb in range(4):
                vown[qb] = vop.tile([P, H, VW], fp8, tag="vown", name="vown")
                nc.gpsimd.memset(vown[qb][:, :, 64:65], 1.0)
                nc.gpsimd.memset(vown[qb][:, :, 65:VW], 0.0)

            def v_sink(qb, ps2):
                dst = vown[qb][:, :, 0:64]
                src = ps2[:].rearrange("p e (a b) -> p (e a) b", a=8)
                if rinvT is not None:
                    nc.vector.tensor_scalar_mul(dst, src, rinvT[:, qb:qb + 1])
                else:
                    nc.vector.tensor_scalar_mul(dst, src, 1.0 / S)
                nc.scalar.dma_start(vview[qb],
                                    vown[qb][:].rearrange("p a b -> p (a b)"))

            proj_tm_dr(WV[blk], rhs8, v_sink)

        rg = [list(range(NC))]

        def emit_ag(blk):
            if timeline:
                nc.gpsimd.dma_start(cc_k_out[blk][0:D, :], cc_k_in[blk][:])
                nc.gpsimd.dma_start(cc_v_out[blk][0:512, :], cc_v_in[blk][:])
            else:
                nc.gpsimd.collective_compute(
                    "AllGather", ALU.bypass, replica_groups=rg,
                    ins=[cc_k_in[blk][:].opt()], outs=[cc_k_out[blk][:].opt()])
                nc.gpsimd.collective_compute(
                    "AllGather", ALU.bypass, replica_groups=rg,
                    ins=[cc_v_in[blk][:].opt()], outs=[cc_v_out[blk][:].opt()])

        # ---------------- attention ----------------
        def attention(blk, q_fm, o8_fm, hook=None):
            with ExitStack() as actx:
                khp = actx.enter_context(tc.tile_pool(name=f"khp{blk}", bufs=3))
                vhp = actx.enter_context(tc.tile_pool(name=f"vhp{blk}", bufs=6))
                ep = actx.enter_context(tc.tile_pool(name=f"ep{blk}", bufs=8))
                kview = cc_k_out[blk][:].rearrange("(s d p) t -> d p s t",
                                                   s=NC, p=P)
                vview = cc_v_out[blk][:].rearrange("(s k p) f -> p s k f",
                                                   s=NC, p=P)
                for dt in range(DT):          # head pair dt -> heads 2dt, 2dt+1
                    kh = khp.tile([P, NC, 512], bf16, tag="kh", name="kh")
                    nc.sync.dma_start(kh[:], kview[dt])
                    if dt == 1 and hook is not None:
                        hook()
                    for b in range(B):
                        vh = vhp.tile([P, NC, 2, 2 * VW], fp8, tag="vh", name="vh")
                        for kq2 in range(2):
                            nc.sync.dma_start(
                                vh[:, :, kq2, :],
                                vview[:, :, 2 * b + kq2,
                                      2 * VW * dt:2 * VW * dt + 2 * VW])
                        for hi in range(2):
                            h = 2 * dt + hi
                            hs = slice(HD * hi, HD * hi + HD)
                            hv = slice(VW * hi, VW * hi + VW)
                            qa = q_fm[hs, dt, 256 * b:256 * b + 256]
                            if blk == 0:
                                qb_ = q_fm[hs, dt, 128 * (2 * b + 1):
                                           128 * (2 * b + 1) + 128]
                                self_attn_bh(kh, hs, vh, hv, qa, qb_, b, h,
                                             o8_fm, ep)
                            else:
                                cross_attn_bh(kh, hs, vh, hv, qa, b, h,
                                              o8_fm, ep)

        def self_attn_bh(kh, hs, vh, hv, qa, qb_, b, h, o8_fm, ep):
            psOa = mmslot((VW, 256))
            first_av = [True]

            def av(lhsT, rhs, cols, last=False):
                nc.tensor.matmul(psOa[:, cols], lhsT, rhs,
                                 start=first_av[0], stop=last,
                                 perf_mode=PM.DoubleRow,
                                 skip_group_check=True)
                first_av[0] = False

            # kq2 = 0 (k chunk j = s): both q-blocks, N=256; mask on left half
            eas = []
            for i in range(2):
                psA = psp2.tile([P, 2, 512], f32, tag="pb2", name="psA")
                ea = ep.tile([P, 4, 256], fp8, tag="ea", name="ea")
                for j in range(2):
                    sl = psA[:, j, :]
                    nc.tensor.matmul(
                        sl.rearrange("p (a b) -> p a b", a=2)[:, :, 0:128],
                        ident_b[:], smk[:, 0, 4 * i + 2 * j:4 * i + 2 * j + 2, :],
                        start=True, stop=False, skip_group_check=True)
                    for t in range(2):
                        nc.tensor.matmul(
                            sl[:, 256 * t:256 * t + 256],
                            kh[hs, 4 * i + 2 * j + t, 256 * b:256 * b + 128],
                            qa, start=False, stop=(t == 1),
                            skip_group_check=True)
                nc.scalar.activation(ea[:].rearrange("p a b -> p (a b)"),
                                     psA[:].rearrange("p a b -> p (a b)"),
                                     AF.Exp)
                eas.append((4 * i, ea))
            # kq2 = 1 (k chunk j = 15-s): right q-block only, N=128
            psB = psp2.tile([P, 2, 512], f32, tag="pb2", name="psB")
            eb = ep.tile([P, 8, P], fp8, tag="ea", name="eb")
            for j in range(2):
                sl = psB[:, j, :]
                nc.tensor.matmul(
                    sl.rearrange("p (a b) -> p a b", a=4), ident_b[:],
                    smk[:, 1, 4 * j:4 * j + 4, :],
                    start=True, stop=False, skip_group_check=True)
                for t in range(4):
                    nc.tensor.matmul(sl[:, 128 * t:128 * t + 128],
                                     kh[hs, 4 * j + t,
                                        256 * b + 128:256 * b + 256],
                                     qb_, start=False, stop=(t == 3),
                                     skip_group_check=True)
            nc.scalar.activation(eb[:].rearrange("p a b -> p (a b)"),
                                 psB[:].rearrange("p a b -> p (a b)"), AF.Exp)
            for base, ea in eas:
                for u in range(2):
                    av(vh[:, base + 2 * u:base + 2 * u + 2, 0, hv],
                       ea[:, 2 * u:2 * u + 2, :], slice(0, 256))
            for u in range(4):
                sidx = 2 * u
                av(vh[:, sidx:sidx + 2, 1, hv], eb[:, 2 * u:2 * u + 2, :],
                   slice(128, 256), last=(sidx == NC - 2))
            finish_attn(psOa, b, h, o8_fm)

        def cross_attn_bh(kh, hs, vh, hv, qa, b, h, o8_fm, ep):
            psO = mmslot((VW, 256))
            for si in range(4):
                psC = psp2.tile([P, 2, 512], f32, tag="pb2", name="psC")
                ec = ep.tile([P, 2, 2, 256], fp8, tag="ea", name="ec")
                for j in range(2):
                    sidx = 2 * si + j
                    sl = psC[:, j, :]
                    nc.tensor.matmul(
                        sl, ident_b[:],
                        posT[:, 2 * sidx:2 * sidx + 2, :].rearrange(
                            "p a b -> p (a b)"),
                        start=True, stop=False, skip_group_check=True)
                    for kq2 in range(2):
                        nc.tensor.matmul(
                            sl[:, 256 * kq2:256 * kq2 + 256],
                            kh[hs, sidx,
                               256 * b + 128 * kq2:256 * b + 128 * kq2 + 128],
                            qa, start=False, stop=(kq2 == 1),
                            skip_group_check=True)
                nc.scalar.activation(ec[:].rearrange("p a b c -> p (a b c)"),
                                     psC[:].rearrange("p a b -> p (a b)"),
                                     AF.Exp)
                for j in range(2):
                    sidx = 2 * si + j
                    nc.tensor.matmul(psO[:], vh[:, sidx, :, hv],
                                     ec[:, j, :, :],
                                     start=(sidx == 0), stop=(sidx == NC - 1),
                                     perf_mode=PM.DoubleRow,
                                     skip_group_check=True)
            finish_attn(psO, b, h, o8_fm)

        def finish_attn(psO, b, h, o8_fm):
            rec = normp.tile([1, 256], f32, tag="rec", name="rec")
            nc.vector.reciprocal(rec[:], psO[64:65, :])
            lb = normp.tile([HD, 256], f32, tag="lb", name="lb")
            nc.gpsimd.partition_broadcast(lb[:], rec[:])
            nc.vector.tensor_mul(
                o8_fm[HD * (h % 2):HD * (h % 2) + HD, h // 2,
                      256 * b:256 * b + 256],
                psO[0:64, :], lb[:])

        # ================= phase 0: rms1, K/V/Q projections, collectives ====
        with ExitStack() as p0:
            q1p = p0.enter_context(tc.tile_pool(name="q1p", bufs=1))
            o1p = p0.enter_context(tc.tile_pool(name="o1p", bufs=1))
            q1_fm = q1p.tile([P, DT, 512], bf16)
            o1_fm = o1p.tile([P, DT, 512], fp8)

            with ExitStack() as pin:
                xinp = pin.enter_context(tc.tile_pool(name="xinp", bufs=1))
                x8 = xinp.tile([P, DT, 512], fp8)
                nc.sync.dma_start(x8[:],
                                  x8_in.ap().rearrange("d p t -> p d t"))
                mem8 = xinp.tile([P, DT, 512], fp8)
                rinv1 = xinp.tile([1, 512], f32)
                nc.gpsimd.dma_start(rinv1[:], rinv1_in.ap())
                rinvT1 = xinp.tile([P, 4], f32)
                nc.gpsimd.dma_start(rinvT1[:], rinvT1_in.ap())
                rb1 = normp.tile([P, 512], f32, tag="rb", bufs=3, name="rb")
                nc.gpsimd.partition_broadcast(rb1[:], rinv1[:])
                nc.gpsimd.dma_start(smk[:], smk_in.ap())
                kv_shard(0, x8, rb1, rinvT1)
                emit_ag(0)

                def q1_sink(dt2, ps2):
                    for e in range(2):
                        nc.vector.scalar_tensor_tensor(
                            q1_fm[:, 2 * dt2 + e, :], ps2[:, e, :], 1.0 / S,
                            rb1[:], op0=ALU.mult, op1=ALU.mult)

                nc.sync.dma_start(mem8[:],
                                  mem8_in.ap().rearrange("d p t -> p d t"))
                proj_fm_dr(WQ[0], x8, q1_sink)
                kv_shard(1, mem8, None, None)
                emit_ag(1)

            # ---- block 1 attention + o-proj + residual
            def defer_x32():
                nc.sync.dma_start(x32[:],
                                  x32_in.ap().rearrange("d p t -> p d t"))

            if stages >= 2:
                attention(0, q1_fm, o1_fm, hook=defer_x32)
            else:
                defer_x32()
            x1 = resid.tile([P, DT, 512], f32, tag="resid", name="x1")

            def o1_sink(dt2, ps2):
                for e in range(2):
                    nc.vector.scalar_tensor_tensor(
                        x1[:, 2 * dt2 + e, :], ps2[:, e, :], 1.0 / S,
                        x32[:, 2 * dt2 + e, :], op0=ALU.mult, op1=ALU.add)

            if stages >= 3:
                proj_fm_dr(WO[0], o1_fm, o1_sink)

        # ================= block 2: cross attention =================
        def block2():
            with ExitStack() as p2:
                q2p = p2.enter_context(tc.tile_pool(name="q2p", bufs=1))
                o2p = p2.enter_context(tc.tile_pool(name="o2p", bufs=1))
                q2_fm = q2p.tile([P, DT, 512], bf16)
                o2_fm = o2p.tile([P, DT, 512], fp8)

                nc.gpsimd.dma_start(posT[:], posT_in.ap())
                rb2 = rms_stats(x1)
                h2_8 = h8p.tile([P, DT, 512], fp8, tag="h8", name="h2_8")
                for d in range(DT):
                    nc.vector.tensor_copy(h2_8[:, d, :], x1[:, d, :])

                def q2_sink(dt2, ps2):
                    for e in range(2):
                        nc.vector.scalar_tensor_tensor(
                            q2_fm[:, 2 * dt2 + e, :], ps2[:, e, :], 1.0 / S,
                            rb2[:], op0=ALU.mult, op1=ALU.mult)

                proj_fm_dr(WQ[1], h2_8, q2_sink)

                if stages >= 5:
                    attention(1, q2_fm, o2_fm)
                x2 = resid.tile([P, DT, 512], f32, tag="resid", name="x2")

                def o2_sink(dt2, ps2):
                    for e in range(2):
                        nc.vector.scalar_tensor_tensor(
                            x2[:, 2 * dt2 + e, :], ps2[:, e, :], 1.0 / S,
                            x1[:, 2 * dt2 + e, :], op0=ALU.mult, op1=ALU.add)

                if stages >= 6:
                    proj_fm_dr(WO[1], o2_fm, o2_sink)
                return x2

        # ================= block 3: FFN =================
        def ffn_proj(w_dram, rhs_hi_of, rhs_lo_of, sink2, dts, kph, wpool):
            """hi-lo fp8 DR: out = Wh@hi + Wh@lo + Wl@hi; weights [Wh;Wl]."""
            for dt2 in range(dts // 2):
                ps2 = psp2.tile([P, 2, 512], f32, tag="pb2", name="ps2")
                wh = wpool.tile([P, 2, kph, 2, P], fp8, tag="wlhs", name="wh")
                nc.sync.dma_start(
                    wh[:], w_dram[2 * dt2:2 * dt2 + 2, :, 0:kph].rearrange(
                        "e p k t m -> p e k t m"))
                wl = wpool.tile([P, 2, kph, 2, P], fp8, tag="wlhs", name="wl")
                nc.sync.dma_start(
                    wl[:], w_dram[2 * dt2:2 * dt2 + 2, :,
                                  kph:2 * kph].rearrange(
                        "e p k t m -> p e k t m"))
                for e in range(2):
                    for kp in range(kph):
                        nc.tensor.matmul(ps2[:, e, :], wh[:, e, kp, :, :],
                                         rhs_hi_of(kp), start=(kp == 0),
                                         stop=False, perf_mode=PM.DoubleRow,
                                         skip_group_check=True)
                    for kp in range(kph):
                        nc.tensor.matmul(ps2[:, e, :], wh[:, e, kp, :, :],
                                         rhs_lo_of(kp), start=False,
                                         stop=False, perf_mode=PM.DoubleRow,
                                         skip_group_check=True)
                for e in range(2):
                    for kp in range(kph):
                        nc.tensor.matmul(ps2[:, e, :], wl[:, e, kp, :, :],
                                         rhs_hi_of(kp), start=False,
                                         stop=(kp == kph - 1),
                                         perf_mode=PM.DoubleRow,
                                         skip_group_check=True)
                sink2(dt2, ps2)

        def block3(x2):
            with ExitStack() as p3:
                zp = p3.enter_context(tc.tile_pool(name="zp", bufs=1))
                zc = zp.tile([P, 2 * KF, 512], fp8)   # [zh(32), zl(32)]
                rb3 = rms_stats(x2)
                h3c = h8p.tile([P, 2 * DT, 512], fp8, tag="h8", name="h3c")
                for d in range(DT):
                    nc.vector.tensor_copy(h3c[:, d, :], x2[:, d, :])
                for d in range(DT):
                    nc.vector.tensor_sub(h3c[:, DT + d, :], x2[:, d, :],
                                         h3c[:, d, :])

                def hh_of(kp):
                    return h3c[:, 2 * kp:2 * kp + 2, :]

                def hl_of(kp):
                    return h3c[:, DT + 2 * kp:DT + 2 * kp + 2, :]

                def z_sink(kf2, ps2):
                    for e in range(2):
                        kf = 2 * kf2 + e
                        nc.scalar.activation(zc[:, kf, :], ps2[:, e, :],
                                             AF.Relu)
                        nc.vector.scalar_tensor_tensor(
                            zc[:, KF + kf, :], ps2[:, e, :], 0.0,
                            zc[:, kf, :], op0=ALU.max, op1=ALU.subtract)

                ffnw = p3.enter_context(tc.tile_pool(name="ffnw", bufs=3))
                ffn_proj(W1, hh_of, hl_of, z_sink, KF, KP, ffnw)

                x3p = p3.enter_context(tc.tile_pool(name="x3p", bufs=3))
                rb3s = normp.tile([P, 512], f32, tag="rb3s", bufs=1,
                                  name="rb3s")
                nc.vector.tensor_scalar_mul(rb3s[:], rb3[:], 1.0 / (S * S))

                def zh_of(kp):
                    return zc[:, 2 * kp:2 * kp + 2, :]

                def zl_of(kp):
                    return zc[:, KF + 2 * kp:KF + 2 * kp + 2, :]

                def x3_sink(dt2, ps2):
                    for e in range(2):
                        x3t = x3p.tile([P, 512], f32, tag="x3t", name="x3t")
                        nc.vector.tensor_mul(x3t[:], ps2[:, e, :], rb3s[:])
                        nc.vector.tensor_add(x3t[:], x3t[:],
                                             x2[:, 2 * dt2 + e, :])
                        nc.scalar.dma_start(out[2 * dt2 + e], x3t[:])

                ffn_proj(W2, zh_of, zl_of, x3_sink, DT, KFP, ffnw)

        if stages >= 4:
            x2 = block2()
            if stages >= 7:
                block3(x2)

    nc.compile()
    return nc


def _get_nc():
    if "nc" not in _CACHE:
        _CACHE["nc"] = _build()
    return _CACHE["nc"]


def _prep_weights(Wq1, Wk1, Wv1, Wo1, Wq2, Wk2, Wv2, Wo2, W1a, W2a,
                  g1, g2, g3):
    import ml_dtypes
    e4 = ml_dtypes.float8_e4m3

    def fm_lhsT(W, dts):
        # [Din, Dout] -> [dt, p, kp, two, m] tiles (contraction pairs)
        din = W.shape[0]
        kpn = din // 256
        a = W.reshape(kpn, 2, P, dts, P)
        return np.ascontiguousarray(a.transpose(3, 2, 0, 1, 4)).astype(e4)

    def hilo_lhsT(W, dts):
        # stacked [Wh, Wl] fp8 contraction blocks in fm_lhsT layout
        Wh = W.astype(e4).astype(np.float32)
        Wl = W - Wh
        ah = fm_lhsT(Wh, dts)
        al = fm_lhsT(Wl, dts)
        return np.ascontiguousarray(np.concatenate([ah, al], axis=2))

    def bf_lhsT(W, dts):
        # [Din, Dout] -> [dt, p, dk, m] bf16 tiles
        din = W.shape[0]
        dkn = din // P
        a = W.reshape(dkn, P, dts, P)
        return np.ascontiguousarray(a.transpose(2, 1, 0, 3)).astype(
            ml_dtypes.bfloat16)

    def tm_rhs(W):
        # [Din, 1024] -> [fh, p, kp, two, n]
        a = W.reshape(KP, 2, P, 2, 512)
        return np.ascontiguousarray(a.transpose(3, 2, 0, 1, 4)).astype(e4)

    gc1 = g1[:, None]
    gc3 = g3[:, None]
    gc2 = g2[:, None]
    return dict(
        wq1=fm_lhsT(S * 0.125 * gc1 * Wq1, DT),
        wk1=fm_lhsT(S * gc1 * Wk1, DT),
        wv1=tm_rhs(S * gc1 * Wv1),
        wo1=fm_lhsT(S * Wo1, DT),
        wq2=fm_lhsT(S * 0.125 * gc2 * Wq2, DT),
        wk2=fm_lhsT(S * Wk2, DT),
        wv2=tm_rhs(S * Wv2),
        wo2=fm_lhsT(S * Wo2, DT),
        w1=hilo_lhsT(S * gc3 * W1a, KF),
        w2=hilo_lhsT(S * W2a, DT),
    )


def _in_maps(x, memory, pos, common):
    import ml_dtypes
    e4 = ml_dtypes.float8_e4m3
    bf = ml_dtypes.bfloat16
    ar = np.arange(P)
    tri = np.where(ar[:, None] > ar[None, :], np.float32(-1e9),
                   np.float32(0.0)).astype(np.float32)
    full = np.full((P, P), np.float32(-1e9), np.float32)
    zero = np.zeros((P, P), np.float32)
    maps = []
    for c in range(NC):
        ch = [c, 15 - c]
        xc = np.concatenate([x[b, ch[q2] * P:(ch[q2] + 1) * P, :]
                             for b, q2 in BLOCKS], axis=0)           # [512, D]
        mc = np.concatenate([memory[b, ch[q2] * P:(ch[q2] + 1) * P, :]
                             for b, q2 in BLOCKS], axis=0)
        x_fm = np.ascontiguousarray(xc.T.reshape(DT, P, 512))
        m_fm = np.ascontiguousarray(mc.T.reshape(DT, P, 512))
        # posT[p, tidx, qi*128+qq] = pos[ch[qi]*128+qq, 128*j+p] - ln64
        posT = np.empty((P, 16, 256), np.float32)
        for j in range(16):
            tidx = 2 * j if j < 8 else 2 * (15 - j) + 1
            for qi in range(2):
                blkp = pos[ch[qi] * P:(ch[qi] + 1) * P, j * P:(j + 1) * P]
                posT[:, tidx, qi * P:(qi + 1) * P] = blkp.T
        posT -= LN64
        sm = np.zeros((P, 2, NC, P), np.float32)
        for s in range(NC):
            sm[:, 0, s, :] = zero if s < c else (tri if s == c else full)
            sm[:, 1, s, :] = zero if s > c else (tri if s == c else full)
        ms = (xc.astype(np.float32) ** 2).mean(-1) + EPS     # [512]
        rinv1 = (1.0 / np.sqrt(ms)).astype(np.float32)
        m = dict(common)
        m.update(x32=x_fm, x8=x_fm.astype(e4), mem8=m_fm.astype(e4),
                 posT=posT.astype(bf), smk=sm.astype(bf),
                 rinv1=rinv1.reshape(1, 512),
                 rinvT1=np.ascontiguousarray(
                     (rinv1 / S).reshape(4, P).T),
                 identb=np.eye(P, dtype=np.float32).astype(bf))
        maps.append(m)
    return maps


def kernel(x, memory, position_embedding, casual_mask,
           g1, Wq1, Wk1, Wv1, Wo1,
           g2, Wq2, Wk2, Wv2, Wo2,
           g3, W1, W2):
    from concourse.bass_utils import run_bass_kernel_spmd

    x = np.asarray(x, np.float32)
    memory = np.asarray(memory, np.float32)
    pos = np.asarray(position_embedding, np.float32).reshape(L, L)
    common = _prep_weights(
        np.asarray(Wq1, np.float32), np.asarray(Wk1, np.float32),
        np.asarray(Wv1, np.float32), np.asarray(Wo1, np.float32),
        np.asarray(Wq2, np.float32), np.asarray(Wk2, np.float32),
        np.asarray(Wv2, np.float32), np.asarray(Wo2, np.float32),
        np.asarray(W1, np.float32), np.asarray(W2, np.float32),
        np.asarray(g1, np.float32), np.asarray(g2, np.float32),
        np.asarray(g3, np.float32))
    nc = _get_nc()
    res = run_bass_kernel_spmd(nc, _in_maps(x, memory, pos, common),
                               core_ids=list(range(NC)))

    outp = np.empty((B, L, D), np.float32)
    for c in range(NC):
        ch = [c, 15 - c]
        o = res.results[c]["out"]          # [DT, P, 512]
        o_t = o.transpose(2, 0, 1).reshape(512, D)   # [tok, D]
        for i, (b, q2) in enumerate(BLOCKS):
            outp[b, ch[q2] * P:(ch[q2] + 1) * P, :] = o_t[i * P:(i + 1) * P]
    return outp
